# revision 12
# baseline (speedup 1.0000x reference)
"""Coupled-map-lattice kernel for Trainium2, data-parallel over 8 NeuronCores.

Reference recurrence (per row n, channels c=0..255, 20 steps):
    mapped = r * g * (1 - g)
    local  = circular 5-tap conv of mapped over c
    glob   = mapped @ W_cc
    g'     = (1-beta)*((1-eps)*mapped + eps*0.5*(local+glob)) + beta*drive
    out    = clip(g_20, 1e-4, 1-1e-4)

Folded form used on device (host precomputes A_neg, qc):
    mapped = r*(1/4 - t),  t = (g - 1/2)^2
    g'     = t @ A_neg + bias2,   bias2 = qc + beta*drive   (constant over steps)
where A[c',c] = (1-beta_c)*[(1-eps_c)*I + eps_c*0.5*(B + W_cc)][c',c],
      B the circulant 5-tap matrix, A_neg = -(r ⊙rows A), qc = 1/4 * (r @ A).

Per-core loop (state transposed: channels on partitions, fp16 matmul operands).
Work is split into three per-column-range "lanes" to balance the engines:
  lane P: PE adds bias via an extra identity matmul; ACT squares from PSUM
  lane M: DVE adds bias (psum+b32); ACT squares from SBUF
  lane V: DVE adds shifted bias (psum+bias-0.5 = u) and squares (u*u) itself
"""

import numpy as np

N, C, KTAPS, STEPS = 131072, 256, 5, 20
N_CORES = 8
N_SHARD = N // N_CORES          # 16384 rows per core
CHUNK = 4096                    # rows resident on-chip per chunk
PSUM_W = 512                    # matmul moving free dim / psum bank width
PSUM_TILE_W = 1024              # psum tile width (2 banks)
LANES = "PPPMMMVM"              # lane of tile index (ci*2*n_ptiles + j*n_ptiles + p) % len

_CACHED_NC = None


def _build_nc():
    import concourse.tile as tile
    from concourse import bacc, mybir

    f32 = mybir.dt.float32
    f16 = mybir.dt.float16
    Act = mybir.ActivationFunctionType
    Alu = mybir.AluOpType

    nc = bacc.Bacc("TRN2", target_bir_lowering=False)
    driveT = nc.declare_dram_parameter("driveT", [C, N_SHARD], f32, isOutput=False)
    a_blk = nc.declare_dram_parameter("a_blk", [128, 640], f32, isOutput=False)
    vecs = nc.declare_dram_parameter("vecs", [128, 6], f32, isOutput=False)
    outT = nc.declare_dram_parameter("outT", [C, N_SHARD], f32, isOutput=True)

    n_chunks = N_SHARD // CHUNK
    n_ptiles = CHUNK // PSUM_TILE_W
    n_sub = PSUM_TILE_W // PSUM_W
    CLIP_LO, CLIP_HI = 1e-4, 1.0 - 1e-4

    with tile.TileContext(nc) as tc:
        with (
            tc.tile_pool(name="const", bufs=1) as constp,
            tc.tile_pool(name="io", bufs=2) as iop,
            tc.tile_pool(name="state", bufs=2) as statep,
            tc.tile_pool(name="psum", bufs=4, space="PSUM") as psump,
        ):
            # ---- constants: A blocks (cols 0-511) + I (cols 512-639), fp16 ----
            a_raw = constp.tile([128, 640], f32)
            nc.gpsimd.dma_start(a_raw[:], a_blk[:])
            a_t = constp.tile([128, 640], f16)
            nc.scalar.copy(a_t[:], a_raw[:])
            v = constp.tile([128, 6], f32)
            nc.gpsimd.dma_start(v[:], vecs[:])
            negh = constp.tile([128, 1], f32)
            nc.vector.memset(negh[:], -0.5)
            posh = constp.tile([128, 1], f32)
            nc.vector.memset(posh[:], 0.5)

            for ci in range(n_chunks):
                col0 = ci * CHUNK

                def lane(j, p):
                    return LANES[(ci * 2 * n_ptiles + j * n_ptiles + p) % len(LANES)]

                d = [iop.tile([128, CHUNK], f32, tag=f"d{j}", name=f"d{j}_{ci}")
                     for j in range(2)]
                for j in range(2):
                    nc.gpsimd.dma_start(
                        d[j][:], driveT[j * 128:(j + 1) * 128, col0:col0 + CHUNK]
                    )
                tA = [statep.tile([128, CHUNK], f16, tag=f"tA{j}", name=f"tA{j}_{ci}")
                      for j in range(2)]
                tB = [statep.tile([128, CHUNK], f16, tag=f"tB{j}", name=f"tB{j}_{ci}")
                      for j in range(2)]
                # one bias tile per (j, ptile): dtype/content depends on lane
                bias = [[None] * n_ptiles, [None] * n_ptiles]
                g = [[None] * n_ptiles, [None] * n_ptiles]
                for j in range(2):
                    for p in range(n_ptiles):
                        ln = lane(j, p)
                        dt_b = f16 if ln == "P" else f32
                        bias[j][p] = statep.tile(
                            [128, PSUM_TILE_W], dt_b, tag=f"bias{j}{p}",
                            name=f"bias{j}{p}_{ci}",
                        )
                        if ln == "V":
                            g[j][p] = statep.tile(
                                [128, PSUM_TILE_W], f32, tag=f"g{j}{p}",
                                name=f"g{j}{p}_{ci}",
                            )

                # t0 = Square(drive - 0.5); per-lane bias tiles
                for j in range(2):
                    nc.scalar.activation(tA[j][:], d[j][:], Act.Square,
                                         bias=negh[:], scale=1.0)
                for j in range(2):
                    for p in range(n_ptiles):
                        ln = lane(j, p)
                        sl = slice(p * PSUM_TILE_W, (p + 1) * PSUM_TILE_W)
                        qcol = (4 + j) if ln == "V" else (2 + j)  # qc-0.5 for lane V
                        nc.vector.tensor_scalar(
                            bias[j][p][:], d[j][:, sl], v[:, j:j + 1],
                            v[:, qcol:qcol + 1], Alu.mult, Alu.add,
                        )

                cur, nxt = tA, tB
                ob = None
                for step in range(STEPS):
                    last = step == STEPS - 1
                    if last:
                        ob = [iop.tile([128, CHUNK], f32, tag=f"d{j}",
                                       name=f"ob{j}_{ci}") for j in range(2)]
                    for j in range(2):
                        for p in range(n_ptiles):
                            ln = lane(j, p)
                            pc0 = p * PSUM_TILE_W
                            ps = psump.tile([128, PSUM_TILE_W], f32, tag="ps",
                                            name=f"ps_{ci}_{step}_{j}_{p}")
                            for s in range(n_sub):
                                sl_p = slice(s * PSUM_W, (s + 1) * PSUM_W)
                                c0 = pc0 + s * PSUM_W
                                sl_c = slice(c0, c0 + PSUM_W)
                                nc.tensor.matmul(
                                    ps[:, sl_p], a_t[:, j * 128:(j + 1) * 128],
                                    cur[0][:, sl_c], start=True, stop=False,
                                )
                                nc.tensor.matmul(
                                    ps[:, sl_p], a_t[:, (2 + j) * 128:(3 + j) * 128],
                                    cur[1][:, sl_c], start=False, stop=ln != "P",
                                )
                                if ln == "P":
                                    nc.tensor.matmul(
                                        ps[:, sl_p], a_t[:, 512:640],
                                        bias[j][p][:, sl_p], start=False, stop=True,
                                    )
                            sl_t = slice(pc0, pc0 + PSUM_TILE_W)
                            if ln == "P":
                                if not last:
                                    nc.scalar.activation(
                                        nxt[j][:, sl_t], ps[:], Act.Square,
                                        bias=negh[:], scale=1.0,
                                    )
                                else:
                                    nc.vector.tensor_scalar(
                                        ob[j][:, sl_t], ps[:],
                                        CLIP_LO, CLIP_HI, Alu.max, Alu.min,
                                    )
                            elif ln == "M":
                                # g' computed in place in PSUM, squared from PSUM
                                nc.vector.tensor_tensor(
                                    ps[:], ps[:], bias[j][p][:], Alu.add
                                )
                                if not last:
                                    nc.scalar.activation(
                                        nxt[j][:, sl_t], ps[:], Act.Square,
                                        bias=negh[:], scale=1.0,
                                    )
                                else:
                                    nc.vector.tensor_scalar(
                                        ob[j][:, sl_t], ps[:],
                                        CLIP_LO, CLIP_HI, Alu.max, Alu.min,
                                    )
                            else:  # lane V: u = psum + (bias2 - 0.5); t' = u*u
                                nc.vector.tensor_tensor(
                                    g[j][p][:], ps[:], bias[j][p][:], Alu.add
                                )
                                if not last:
                                    nc.vector.tensor_tensor(
                                        nxt[j][:, sl_t], g[j][p][:], g[j][p][:],
                                        Alu.mult,
                                    )
                                else:
                                    nc.vector.tensor_scalar(
                                        g[j][p][:], g[j][p][:],
                                        CLIP_LO - 0.5, CLIP_HI - 0.5,
                                        Alu.max, Alu.min,
                                    )
                                    nc.scalar.activation(
                                        ob[j][:, sl_t], g[j][p][:], Act.Identity,
                                        bias=posh[:], scale=1.0,
                                    )
                    cur, nxt = nxt, cur

                for j in range(2):
                    nc.gpsimd.dma_start(
                        outT[j * 128:(j + 1) * 128, col0:col0 + CHUNK], ob[j][:]
                    )
    nc.compile()
    return nc


def _get_nc():
    global _CACHED_NC
    if _CACHED_NC is None:
        _CACHED_NC = _build_nc()
    return _CACHED_NC


def _fold_constants(r, eps, beta, K_local, W_cc):
    """Host-side fold of the per-step linear operator into A_neg / qc."""
    pad = KTAPS // 2
    cp = np.arange(C)[:, None]
    c = np.arange(C)[None, :]
    j = (cp - c + pad) % C
    B = np.where(j < KTAPS, K_local.astype(np.float64)[np.minimum(j, KTAPS - 1)], 0.0)
    A = (1.0 - beta.astype(np.float64))[None, :] * (
        (1.0 - eps.astype(np.float64))[None, :] * np.eye(C)
        + eps.astype(np.float64)[None, :] * 0.5 * (B + W_cc.astype(np.float64))
    )
    A_r = r.astype(np.float64)[:, None] * A
    A_neg = (-A_r).astype(np.float32)          # [C, C]; g' = t @ A_neg + bias2
    qc = (0.25 * A_r.sum(axis=0)).astype(np.float32)   # [C]
    return A_neg, qc


def _pack_inputs(drive, r, eps, beta, K_local, W_cc):
    A_neg, qc = _fold_constants(r, eps, beta, K_local, W_cc)
    # lhsT blocks laid out [k0m0 | k0m1 | k1m0 | k1m1 | I]:
    # matmul for output tile m uses cols m*128 (k=0) and (2+m)*128 (k=1)
    blocks = [A_neg[k * 128:(k + 1) * 128, m * 128:(m + 1) * 128]
              for k in range(2) for m in range(2)]
    blocks.append(np.eye(128, dtype=np.float32))
    a_blk = np.concatenate(blocks, axis=1).astype(np.float32)   # [128, 640]
    qcs = qc - np.float32(0.5)
    vecs = np.stack(
        [beta[0:128], beta[128:256], qc[0:128], qc[128:256], qcs[0:128], qcs[128:256]],
        axis=1,
    ).astype(np.float32)                       # [128, 6]
    driveT = np.ascontiguousarray(drive.T.astype(np.float32))   # [C, N]
    in_maps = []
    for i in range(N_CORES):
        shard = np.ascontiguousarray(driveT[:, i * N_SHARD:(i + 1) * N_SHARD])
        in_maps.append({"driveT": shard, "a_blk": a_blk, "vecs": vecs})
    return in_maps


def run(drive, r, eps, beta, K_local, W_cc, trace=False, trace_kwargs=None):
    from concourse.bass_utils import run_bass_kernel_spmd

    nc = _get_nc()
    in_maps = _pack_inputs(drive, r, eps, beta, K_local, W_cc)
    res = run_bass_kernel_spmd(
        nc, in_maps, core_ids=list(range(N_CORES)),
        trace=trace, **(trace_kwargs or {}),
    )
    outT = np.concatenate([res.results[i]["outT"] for i in range(N_CORES)], axis=1)
    out = np.ascontiguousarray(outT.T).astype(np.float32)
    return out, res


def kernel(drive, r, eps, beta, K_local, W_cc):
    out, _ = run(
        np.asarray(drive), np.asarray(r), np.asarray(eps), np.asarray(beta),
        np.asarray(K_local), np.asarray(W_cc),
    )
    return out


# revision 13
# speedup vs baseline: 1.0231x; 1.0231x over previous
"""Coupled-map-lattice kernel for Trainium2, data-parallel over 8 NeuronCores.

Reference recurrence (per row n, channels c=0..255, 20 steps):
    mapped = r * g * (1 - g)
    local  = circular 5-tap conv of mapped over c
    glob   = mapped @ W_cc
    g'     = (1-beta)*((1-eps)*mapped + eps*0.5*(local+glob)) + beta*drive
    out    = clip(g_20, 1e-4, 1-1e-4)

Folded form used on device (host precomputes A_neg, qc):
    mapped = r*(1/4 - t),  t = (g - 1/2)^2
    g'     = t @ A_neg + bias2,   bias2 = qc + beta*drive   (constant over steps)
where A[c',c] = (1-beta_c)*[(1-eps_c)*I + eps_c*0.5*(B + W_cc)][c',c],
      B the circulant 5-tap matrix, A_neg = -(r ⊙rows A), qc = 1/4 * (r @ A).

Per-core loop (state transposed: channels on partitions, fp16 matmul operands).
Work is split into three per-column-range "lanes" to balance the engines:
  lane P: PE adds bias via an extra identity matmul; ACT squares from PSUM
  lane M: DVE adds bias (psum+b32); ACT squares from SBUF
  lane V: DVE adds shifted bias (psum+bias-0.5 = u) and squares (u*u) itself
"""

import numpy as np

N, C, KTAPS, STEPS = 131072, 256, 5, 20
N_CORES = 8
N_SHARD = N // N_CORES          # 16384 rows per core
CHUNK = 4096                    # rows resident on-chip per chunk
PSUM_W = 512                    # matmul moving free dim / psum bank width
PSUM_TILE_W = 1024              # psum tile width (2 banks)
LANES = "PPPMMMMM"              # lane of tile index (ci*2*n_ptiles + j*n_ptiles + p) % len

_CACHED_NC = None


def _build_nc():
    import concourse.tile as tile
    from concourse import bacc, mybir

    f32 = mybir.dt.float32
    f16 = mybir.dt.float16
    Act = mybir.ActivationFunctionType
    Alu = mybir.AluOpType

    nc = bacc.Bacc("TRN2", target_bir_lowering=False)
    driveT = nc.declare_dram_parameter("driveT", [C, N_SHARD], f32, isOutput=False)
    a_blk = nc.declare_dram_parameter("a_blk", [128, 640], f32, isOutput=False)
    vecs = nc.declare_dram_parameter("vecs", [128, 6], f32, isOutput=False)
    outT = nc.declare_dram_parameter("outT", [C, N_SHARD], f32, isOutput=True)

    n_chunks = N_SHARD // CHUNK
    n_ptiles = CHUNK // PSUM_TILE_W
    n_sub = PSUM_TILE_W // PSUM_W
    CLIP_LO, CLIP_HI = 1e-4, 1.0 - 1e-4

    with tile.TileContext(nc) as tc:
        with (
            tc.tile_pool(name="const", bufs=1) as constp,
            tc.tile_pool(name="io", bufs=2) as iop,
            tc.tile_pool(name="state", bufs=2) as statep,
            tc.tile_pool(name="psum", bufs=4, space="PSUM") as psump,
        ):
            # ---- constants: A blocks (cols 0-511) + I (cols 512-639), fp16 ----
            a_raw = constp.tile([128, 640], f32)
            nc.gpsimd.dma_start(a_raw[:], a_blk[:])
            a_t = constp.tile([128, 640], f16)
            nc.scalar.copy(a_t[:], a_raw[:])
            v = constp.tile([128, 6], f32)
            nc.gpsimd.dma_start(v[:], vecs[:])
            negh = constp.tile([128, 1], f32)
            nc.vector.memset(negh[:], -0.5)
            posh = constp.tile([128, 1], f32)
            nc.vector.memset(posh[:], 0.5)

            for ci in range(n_chunks):
                col0 = ci * CHUNK

                def lane(j, p):
                    return LANES[(ci * 2 * n_ptiles + j * n_ptiles + p) % len(LANES)]

                d = [iop.tile([128, CHUNK], f32, tag=f"d{j}", name=f"d{j}_{ci}")
                     for j in range(2)]
                for j in range(2):
                    nc.gpsimd.dma_start(
                        d[j][:], driveT[j * 128:(j + 1) * 128, col0:col0 + CHUNK]
                    )
                tA = [statep.tile([128, CHUNK], f16, tag=f"tA{j}", name=f"tA{j}_{ci}")
                      for j in range(2)]
                tB = [statep.tile([128, CHUNK], f16, tag=f"tB{j}", name=f"tB{j}_{ci}")
                      for j in range(2)]
                # one bias tile per (j, ptile): dtype/content depends on lane
                bias = [[None] * n_ptiles, [None] * n_ptiles]
                g = [[None] * n_ptiles, [None] * n_ptiles]
                for j in range(2):
                    for p in range(n_ptiles):
                        ln = lane(j, p)
                        dt_b = f16 if ln == "P" else f32
                        bias[j][p] = statep.tile(
                            [128, PSUM_TILE_W], dt_b, tag=f"bias{j}{p}",
                            name=f"bias{j}{p}_{ci}",
                        )
                        if ln == "V":
                            g[j][p] = statep.tile(
                                [128, PSUM_TILE_W], f32, tag=f"g{j}{p}",
                                name=f"g{j}{p}_{ci}",
                            )

                # t0 = Square(drive - 0.5); per-lane bias tiles
                for j in range(2):
                    nc.scalar.activation(tA[j][:], d[j][:], Act.Square,
                                         bias=negh[:], scale=1.0)
                for j in range(2):
                    for p in range(n_ptiles):
                        ln = lane(j, p)
                        sl = slice(p * PSUM_TILE_W, (p + 1) * PSUM_TILE_W)
                        qcol = (4 + j) if ln == "V" else (2 + j)  # qc-0.5 for lane V
                        nc.vector.tensor_scalar(
                            bias[j][p][:], d[j][:, sl], v[:, j:j + 1],
                            v[:, qcol:qcol + 1], Alu.mult, Alu.add,
                        )

                cur, nxt = tA, tB
                ob = None
                for step in range(STEPS):
                    last = step == STEPS - 1
                    if last:
                        ob = [iop.tile([128, CHUNK], f32, tag=f"d{j}",
                                       name=f"ob{j}_{ci}") for j in range(2)]
                    for j in range(2):
                        for p in range(n_ptiles):
                            ln = lane(j, p)
                            pc0 = p * PSUM_TILE_W
                            ps = psump.tile([128, PSUM_TILE_W], f32, tag="ps",
                                            name=f"ps_{ci}_{step}_{j}_{p}")
                            for s in range(n_sub):
                                sl_p = slice(s * PSUM_W, (s + 1) * PSUM_W)
                                c0 = pc0 + s * PSUM_W
                                sl_c = slice(c0, c0 + PSUM_W)
                                nc.tensor.matmul(
                                    ps[:, sl_p], a_t[:, j * 128:(j + 1) * 128],
                                    cur[0][:, sl_c], start=True, stop=False,
                                )
                                nc.tensor.matmul(
                                    ps[:, sl_p], a_t[:, (2 + j) * 128:(3 + j) * 128],
                                    cur[1][:, sl_c], start=False, stop=ln != "P",
                                )
                                if ln == "P":
                                    nc.tensor.matmul(
                                        ps[:, sl_p], a_t[:, 512:640],
                                        bias[j][p][:, sl_p], start=False, stop=True,
                                    )
                            sl_t = slice(pc0, pc0 + PSUM_TILE_W)
                            if ln == "P":
                                if not last:
                                    nc.scalar.activation(
                                        nxt[j][:, sl_t], ps[:], Act.Square,
                                        bias=negh[:], scale=1.0,
                                    )
                                else:
                                    nc.vector.tensor_scalar(
                                        ob[j][:, sl_t], ps[:],
                                        CLIP_LO, CLIP_HI, Alu.max, Alu.min,
                                    )
                            elif ln == "M":
                                # g' computed in place in PSUM, squared from PSUM
                                nc.vector.tensor_tensor(
                                    ps[:], ps[:], bias[j][p][:], Alu.add
                                )
                                if not last:
                                    nc.scalar.activation(
                                        nxt[j][:, sl_t], ps[:], Act.Square,
                                        bias=negh[:], scale=1.0,
                                    )
                                else:
                                    nc.vector.tensor_scalar(
                                        ob[j][:, sl_t], ps[:],
                                        CLIP_LO, CLIP_HI, Alu.max, Alu.min,
                                    )
                            else:  # lane V: u = psum + (bias2 - 0.5); t' = u*u
                                nc.vector.tensor_tensor(
                                    g[j][p][:], ps[:], bias[j][p][:], Alu.add
                                )
                                if not last:
                                    nc.vector.tensor_tensor(
                                        nxt[j][:, sl_t], g[j][p][:], g[j][p][:],
                                        Alu.mult,
                                    )
                                else:
                                    nc.vector.tensor_scalar(
                                        g[j][p][:], g[j][p][:],
                                        CLIP_LO - 0.5, CLIP_HI - 0.5,
                                        Alu.max, Alu.min,
                                    )
                                    nc.scalar.activation(
                                        ob[j][:, sl_t], g[j][p][:], Act.Identity,
                                        bias=posh[:], scale=1.0,
                                    )
                    cur, nxt = nxt, cur

                for j in range(2):
                    nc.gpsimd.dma_start(
                        outT[j * 128:(j + 1) * 128, col0:col0 + CHUNK], ob[j][:]
                    )
    nc.compile()
    return nc


def _get_nc():
    global _CACHED_NC
    if _CACHED_NC is None:
        _CACHED_NC = _build_nc()
    return _CACHED_NC


def _fold_constants(r, eps, beta, K_local, W_cc):
    """Host-side fold of the per-step linear operator into A_neg / qc."""
    pad = KTAPS // 2
    cp = np.arange(C)[:, None]
    c = np.arange(C)[None, :]
    j = (cp - c + pad) % C
    B = np.where(j < KTAPS, K_local.astype(np.float64)[np.minimum(j, KTAPS - 1)], 0.0)
    A = (1.0 - beta.astype(np.float64))[None, :] * (
        (1.0 - eps.astype(np.float64))[None, :] * np.eye(C)
        + eps.astype(np.float64)[None, :] * 0.5 * (B + W_cc.astype(np.float64))
    )
    A_r = r.astype(np.float64)[:, None] * A
    A_neg = (-A_r).astype(np.float32)          # [C, C]; g' = t @ A_neg + bias2
    qc = (0.25 * A_r.sum(axis=0)).astype(np.float32)   # [C]
    return A_neg, qc


def _pack_inputs(drive, r, eps, beta, K_local, W_cc):
    A_neg, qc = _fold_constants(r, eps, beta, K_local, W_cc)
    # lhsT blocks laid out [k0m0 | k0m1 | k1m0 | k1m1 | I]:
    # matmul for output tile m uses cols m*128 (k=0) and (2+m)*128 (k=1)
    blocks = [A_neg[k * 128:(k + 1) * 128, m * 128:(m + 1) * 128]
              for k in range(2) for m in range(2)]
    blocks.append(np.eye(128, dtype=np.float32))
    a_blk = np.concatenate(blocks, axis=1).astype(np.float32)   # [128, 640]
    qcs = qc - np.float32(0.5)
    vecs = np.stack(
        [beta[0:128], beta[128:256], qc[0:128], qc[128:256], qcs[0:128], qcs[128:256]],
        axis=1,
    ).astype(np.float32)                       # [128, 6]
    driveT = np.ascontiguousarray(drive.T.astype(np.float32))   # [C, N]
    in_maps = []
    for i in range(N_CORES):
        shard = np.ascontiguousarray(driveT[:, i * N_SHARD:(i + 1) * N_SHARD])
        in_maps.append({"driveT": shard, "a_blk": a_blk, "vecs": vecs})
    return in_maps


def run(drive, r, eps, beta, K_local, W_cc, trace=False, trace_kwargs=None):
    from concourse.bass_utils import run_bass_kernel_spmd

    nc = _get_nc()
    in_maps = _pack_inputs(drive, r, eps, beta, K_local, W_cc)
    res = run_bass_kernel_spmd(
        nc, in_maps, core_ids=list(range(N_CORES)),
        trace=trace, **(trace_kwargs or {}),
    )
    outT = np.concatenate([res.results[i]["outT"] for i in range(N_CORES)], axis=1)
    out = np.ascontiguousarray(outT.T).astype(np.float32)
    return out, res


def kernel(drive, r, eps, beta, K_local, W_cc):
    out, _ = run(
        np.asarray(drive), np.asarray(r), np.asarray(eps), np.asarray(beta),
        np.asarray(K_local), np.asarray(W_cc),
    )
    return out
